# revision 6
# baseline (speedup 1.0000x reference)
"""GCNConv (DGL GraphConv norm='both') on 8 Trainium2 NeuronCores — v2.

out = D_dst^-1/2 * A * (D_src^-1/2 * X * W) + b
  X: [100000, 32] f32, edge_index: [2, 1600000] (src, dst), W: [32, 32], b: [32]

v2 design (vs v1 baseline):
  - NO collective: aggregation is linear, so aggregate RAW scaled features
    x_hat = x * outdeg^-1/2 (f16) and apply W AFTER aggregation. Every core
    computes the full x_hat table (x load is cheap) into its OWN dram — the
    283us AllGather is gone, as are phase-1 transposes/matmuls.
  - Compact f16 message table [100000, 32] (64B rows). dma_gather elements
    must be 256B, so gather fetches node QUADS (4 rows); edges are bucketed
    by (dst window, src%4) so each 128-edge block reads its message at a
    static 32-column slice of the quad.
  - One merged gather stream per span (not per group) -> fewer SWDGE calls.
  - One-hot build split DVE/Pool to balance engine load.
  - Transform-last per window: agg[128,32] -(x nd, ACT)-> f16 -(PE transpose)
    -> [32,128] -(x W, PE)-> psum, bias on DVE, batched DMA of 4 windows.
  - Output written transposed [32, 12544]; host untransposes (layout only).
"""

import os
import sys

import numpy as np

for _p in ("/opt/trn_rl_repo", "/root/.axon_site/_ro/trn_rl_repo"):
    if os.path.isdir(_p) and _p not in sys.path:
        sys.path.insert(0, _p)

N_NODES = 100000
N_CORES = 8
NPC = N_NODES // N_CORES  # 12500
DIN = 32
DOUT = 32
P = 128
NTILE = (NPC + P - 1) // P  # 98 dst windows/core
NPAD = NTILE * P            # 12544
NG = 4                      # src mod-4 groups
QN = N_NODES // NG          # 25000 quads

GMAIN = 781                 # nodes per partition in phase-1 main region
NMAIN = P * GMAIN           # 99968
NTAIL = N_NODES - NMAIN     # 32
GCH = 52                    # phase-1 chunk cols (11 chunks: 10*71+71=781)

SPAN_W = 8                  # windows per gather span


def _build_program(RQ):
    from concourse import bacc, bass, mybir, tile

    f32 = mybir.dt.float32
    f16 = mybir.dt.float16
    i16 = mybir.dt.int16
    i32 = mybir.dt.int32
    Alu = mybir.AluOpType
    Act = mybir.ActivationFunctionType

    WSL = NG * RQ               # slots (128-edge blocks) per window
    SLOTS = NTILE * WSL
    # SBUF-bounded gather span: 8 windows at RQ=5 (tuned); fewer if RQ grows
    span_w = max(2, (SPAN_W * 20) // WSL)
    nspan = (NTILE + span_w - 1) // span_w

    nc = bacc.Bacc(
        "TRN2",
        target_bir_lowering=False,
        debug=False,
        enable_asserts=False,
        num_devices=N_CORES,
    )

    # ---- I/O ----
    x_in = nc.dram_tensor("x_in", [N_NODES, DIN], f32, kind="ExternalInput")
    w_in = nc.dram_tensor("w_in", [DIN, DOUT], f32, kind="ExternalInput")
    b_in = nc.dram_tensor("b_in", [DOUT, 1], f32, kind="ExternalInput")
    qidx = nc.dram_tensor("qidx", [P, SLOTS * 8], i16, kind="ExternalInput")
    dstloc = nc.dram_tensor("dstloc", [P, SLOTS], f16, kind="ExternalInput")
    odeg_in = nc.dram_tensor("odeg_in", [P, GMAIN + 1], i32, kind="ExternalInput")
    ideg_in = nc.dram_tensor("ideg_in", [P, NTILE], i32, kind="ExternalInput")
    iota_in = nc.dram_tensor("iota_in", [P, P], f16, kind="ExternalInput")
    ident_in = nc.dram_tensor("ident_in", [P, P], f32, kind="ExternalInput")
    outT = nc.dram_tensor("outT", [DOUT, NPAD], f16, kind="ExternalOutput")

    # own-DRAM message table (f16, node-major rows of 32)
    m_dram = nc.dram_tensor("m_dram", [N_NODES * DIN], f16, kind="Internal")

    with tile.TileContext(nc) as tc:
        with (
            tc.tile_pool(name="const", bufs=1) as cpool,
            tc.tile_pool(name="p1", bufs=4) as p1pool,
            tc.tile_pool(name="work", bufs=3) as wpool,
            tc.tile_pool(name="nrm", bufs=1) as npool,
            tc.tile_pool(name="ohp", bufs=3) as ohpool,
            tc.tile_pool(name="gath", bufs=2) as gpool,
            tc.tile_pool(name="psA", bufs=4, space="PSUM") as ppa,
            tc.tile_pool(name="psB", bufs=2, space="PSUM") as ppb,
            tc.tile_pool(name="psC", bufs=2, space="PSUM") as ppc,
        ):
            # ---- degree tables first: norms ready before phase-1 ----
            od_t = cpool.tile([P, GMAIN + 1], i32)
            nc.sync.dma_start(out=od_t[:], in_=odeg_in[:])
            id_t = cpool.tile([P, NTILE], i32)
            nc.sync.dma_start(out=id_t[:], in_=ideg_in[:])

            ns_all = cpool.tile([P, GMAIN + 1], f32)  # outdeg^-1/2 (phase-1 layout)
            odf = npool.tile([P, GMAIN + 1], f32, tag="odf")
            nc.vector.tensor_copy(out=odf[:], in_=od_t[:])
            nc.vector.tensor_scalar_max(out=odf[:], in0=odf[:], scalar1=1.0)
            osq = npool.tile([P, GMAIN + 1], f32, tag="osq")
            nc.scalar.activation(out=osq[:], in_=odf[:], func=Act.Sqrt)
            nc.vector.reciprocal(out=ns_all[:], in_=osq[:])

            nd_all = cpool.tile([P, NTILE], f32)  # indeg^-1/2 (dst-lane layout)
            idf = npool.tile([P, NTILE], f32, tag="idf")
            nc.vector.tensor_copy(out=idf[:], in_=id_t[:])
            nc.vector.tensor_scalar_max(out=idf[:], in0=idf[:], scalar1=1.0)
            isq = npool.tile([P, NTILE], f32, tag="isq")
            nc.scalar.activation(out=isq[:], in_=idf[:], func=Act.Sqrt)
            nc.vector.reciprocal(out=nd_all[:], in_=isq[:])

            # ---- constants ----
            qidx_t = cpool.tile([P, SLOTS * 8], i16)
            dst_t = cpool.tile([P, SLOTS], f16)
            nc.sync.dma_start(out=dst_t[:], in_=dstloc[:])
            iota_t = cpool.tile([P, P], f16)
            nc.sync.dma_start(out=iota_t[:], in_=iota_in[:])
            ident_t = cpool.tile([P, P], f32)
            nc.sync.dma_start(out=ident_t[:], in_=ident_in[:])
            wf_t = cpool.tile([DIN, DOUT], f32)
            nc.sync.dma_start(out=wf_t[:], in_=w_in[:])
            w16 = cpool.tile([DIN, DOUT], f16)
            nc.scalar.activation(out=w16[:], in_=wf_t[:], func=Act.Copy)
            b_t = cpool.tile([DOUT, 1], f32)
            nc.sync.dma_start(out=b_t[:], in_=b_in[:])

            # ---- phase 1: x_hat = x * ns -> f16 table, replicated ----
            x_main = x_in[0:NMAIN, :].rearrange("(p g) c -> p g c", p=P)
            m_main = m_dram[0:NMAIN * DIN].rearrange(
                "(p g c) -> p g c", p=P, g=GMAIN
            )
            # tail 32 nodes first (keeps the final m write off the
            # phase-1 -> phase-2 critical path)
            xs2 = p1pool.tile([NTAIL, DIN], f32, tag="xs2")
            nc.sync.dma_start(out=xs2[:], in_=x_in[NMAIN:N_NODES, :])
            xh2 = p1pool.tile([NTAIL, DIN], f16, tag="xh2")
            nc.vector.tensor_tensor(
                out=xh2[:], in0=xs2[:],
                in1=ns_all[0:NTAIL, GMAIN:GMAIN + 1].to_broadcast([NTAIL, DIN]),
                op=Alu.mult,
            )
            nc.sync.dma_start(
                out=m_dram[NMAIN * DIN:].rearrange("(p c) -> p c", p=NTAIL),
                in_=xh2[:],
            )
            n_chunks = (GMAIN + GCH - 1) // GCH
            qsl = (SLOTS * 8 + n_chunks - 1) // n_chunks
            for ci, c0 in enumerate(range(0, GMAIN, GCH)):
                # slice of the big index table: fills DMA bubbles while the
                # phase-1 write waits on the DVE scale
                q0 = ci * qsl
                q1 = min(q0 + qsl, SLOTS * 8)
                if q0 < q1:
                    nc.sync.dma_start(out=qidx_t[:, q0:q1], in_=qidx[:, q0:q1])
                ncg = min(GCH, GMAIN - c0)
                xs = p1pool.tile([P, GCH, DIN], f32, tag="xs")
                nc.sync.dma_start(out=xs[:, :ncg, :], in_=x_main[:, c0:c0 + ncg, :])
                xh = p1pool.tile([P, GCH, DIN], f16, tag="xh")
                nc.vector.tensor_tensor(
                    out=xh[:, :ncg, :], in0=xs[:, :ncg, :],
                    in1=ns_all[:, c0:c0 + ncg].unsqueeze(2)
                    .to_broadcast([P, ncg, DIN]),
                    op=Alu.mult,
                )
                nc.sync.dma_start(out=m_main[:, c0:c0 + ncg, :], in_=xh[:, :ncg, :])

            # ---- phase 2 ----
            m_q = m_dram[:].rearrange("(q e) -> q e", e=P)  # [25000, 128] f16

            q_tiles = [None] * nspan

            def ensure_span(sp):
                if q_tiles[sp] is not None:
                    return
                w0 = sp * span_w
                nw = min(span_w, NTILE - w0)
                qt = gpool.tile([P, span_w * WSL, P], f16, tag="qt")
                # chunk gathers: keep num_idxs within the SWDGE desc ring;
                # last span goes window-by-window to shorten the tail drain
                step = 1 if sp == nspan - 1 else 2
                for wc in range(0, nw, step):
                    nwc = min(step, nw - wc)
                    s0 = (w0 + wc) * WSL
                    nsl = nwc * WSL
                    n_idx = nsl * P
                    nc.gpsimd.dma_gather(
                        out_ap=qt[:, wc * WSL:wc * WSL + nsl, :],
                        in_ap=m_q[:],
                        idxs_ap=qidx_t[:, s0 * 8:(s0 + nsl) * 8],
                        num_idxs=n_idx,
                        num_idxs_reg=n_idx,
                        elem_size=P,
                        single_packet=False,
                    )
                q_tiles[sp] = qt

            LOOKAHEAD = 2
            oh_pending = {}

            def emit_onehot(wn):
                # one-hot [e-lane, slot, d]; sentinel dst=128 matches nothing
                oht = ohpool.tile([P, WSL, P], f16, tag="oh")
                nc.vector.tensor_tensor(
                    out=oht[:],
                    in0=iota_t[:].unsqueeze(1).to_broadcast([P, WSL, P]),
                    in1=dst_t[:, wn * WSL:(wn + 1) * WSL]
                    .unsqueeze(2).to_broadcast([P, WSL, P]),
                    op=Alu.is_equal,
                )
                oh_pending[wn] = oht

            for wn in range(min(LOOKAHEAD, NTILE)):
                emit_onehot(wn)

            for w in range(NTILE):
                sp, wo = divmod(w, span_w)
                ensure_span(sp)
                if sp + 1 < nspan and wo == 0:
                    ensure_span(sp + 1)
                if w + LOOKAHEAD < NTILE:
                    emit_onehot(w + LOOKAHEAD)
                oh = oh_pending.pop(w)
                ps = ppa.tile([P, DOUT], f32)
                qt = q_tiles[sp]
                for s in range(WSL):
                    g = s // RQ
                    nc.tensor.matmul(
                        out=ps[:],
                        lhsT=oh[:, s, :],
                        rhs=qt[:, wo * WSL + s, g * DIN:(g + 1) * DIN],
                        start=(s == 0),
                        stop=(s == WSL - 1),
                    )
                if wo == span_w - 1 or w == NTILE - 1:
                    q_tiles[sp] = None

                # scale by nd, cast f32 (ACT), transpose (PE), cast f16
                agg_sb = wpool.tile([P, DOUT], f32, tag="agg")
                nc.scalar.activation(
                    out=agg_sb[:], in_=ps[:], func=Act.Copy,
                    scale=nd_all[:, w:w + 1],
                )
                pst = ppb.tile([DOUT, P], f32)
                nc.tensor.transpose(out=pst[:], in_=agg_sb[:], identity=ident_t[:])
                aggT = wpool.tile([DOUT, P], f16, tag="aggT")
                nc.scalar.activation(out=aggT[:], in_=pst[:], func=Act.Copy)

                # resT[c',d] = sum_c W[c,c'] aggT[c,d]
                r4 = w % 4
                if r4 == 0:
                    res4 = ppc.tile([DOUT, 4, P], f32)
                nc.tensor.matmul(
                    out=res4[:, r4, :], lhsT=w16[:], rhs=aggT[:],
                    start=True, stop=True,
                )
                if r4 == 3 or w == NTILE - 1:
                    nb = r4 + 1
                    res_sb = wpool.tile([DOUT, 4, P], f16, tag="res")
                    nc.vector.tensor_tensor(
                        out=res_sb[:, :nb, :],
                        in0=res4[:, :nb, :],
                        in1=b_t[:].unsqueeze(2).to_broadcast([DOUT, nb, P]),
                        op=Alu.add,
                    )
                    w0 = w - nb + 1
                    nc.sync.dma_start(
                        out=outT[:, w0 * P:(w + 1) * P].rearrange(
                            "c (n p) -> c n p", p=P
                        ),
                        in_=res_sb[:, :nb, :],
                    )

    nc.compile()
    return nc


def _preprocess(x, edge_index, W, b):
    """Host-side: integer bucketing/sorting only (+ layout copies)."""
    src = np.asarray(edge_index[0], dtype=np.int64)
    dst = np.asarray(edge_index[1], dtype=np.int64)
    x = np.asarray(x, dtype=np.float32)
    W = np.asarray(W, dtype=np.float32)
    b = np.asarray(b, dtype=np.float32)

    core_of = dst // NPC
    per_core = []
    rq_needed = 1
    for k in range(N_CORES):
        sel = core_of == k
        s_k = src[sel]
        d_k = dst[sel] - k * NPC
        win = d_k >> 7
        grp = s_k & 3
        order = np.lexsort((s_k, grp, win))
        s_k = s_k[order]
        d_k = d_k[order]
        wg = win[order] * NG + grp[order]
        wg_counts = np.bincount(wg, minlength=NTILE * NG)
        rq_needed = max(rq_needed, int(np.ceil(wg_counts.max() / P)))
        ideg = np.bincount(d_k, minlength=NPC)
        per_core.append((s_k, d_k, wg_counts, ideg))

    RQ = int(rq_needed)
    WSL = NG * RQ
    SLOTS = NTILE * WSL

    odeg_full = np.bincount(src, minlength=N_NODES).astype(np.int64)
    odeg_arr = np.zeros((P, GMAIN + 1), dtype=np.int32)
    odeg_arr[:, :GMAIN] = odeg_full[:NMAIN].reshape(P, GMAIN)
    odeg_arr[:NTAIL, GMAIN] = odeg_full[NMAIN:]

    iota_rep = np.broadcast_to(
        np.arange(P, dtype=np.float16)[None, :], (P, P)
    ).copy()
    ident = np.eye(P, dtype=np.float32)

    in_maps = []
    for k in range(N_CORES):
        s_k, d_k, wg_counts, ideg = per_core[k]
        wg_starts = np.concatenate([[0], np.cumsum(wg_counts)])[:-1]
        n_e = len(s_k)
        pos = np.arange(n_e) - np.repeat(wg_starts, wg_counts)
        wv = np.repeat(np.arange(NTILE * NG) // NG, wg_counts)
        gv = np.repeat(np.arange(NTILE * NG) % NG, wg_counts)
        js = (wv * WSL + gv * RQ) * P + pos  # stream position, lane-fastest

        qflat = np.zeros(SLOTS * P, dtype=np.int16)
        qflat[js] = (s_k >> 2).astype(np.int16)
        dflat = np.full(SLOTS * P, P, dtype=np.float16)
        dflat[js] = (d_k & 127).astype(np.float16)

        qi = qflat.reshape(SLOTS * P // 16, 16).T  # [16, SLOTS*8]
        qidx_arr = np.tile(qi, (8, 1))
        dst_arr = dflat.reshape(SLOTS, P).T.copy()

        ideg_pad = np.zeros(NPAD, dtype=np.int32)
        ideg_pad[:NPC] = ideg
        ideg_arr = ideg_pad.reshape(NTILE, P).T.copy()

        in_maps.append({
            "x_in": x, "w_in": W, "b_in": b[:, None].copy(),
            "qidx": qidx_arr, "dstloc": dst_arr,
            "odeg_in": odeg_arr, "ideg_in": ideg_arr,
            "iota_in": iota_rep, "ident_in": ident,
        })

    return in_maps, RQ


_prog_cache = {}
_last_results = None


def kernel(x, edge_index, W, b):
    from concourse import bass_utils

    in_maps, RQ = _preprocess(x, edge_index, W, b)
    if RQ not in _prog_cache:
        _prog_cache[RQ] = _build_program(RQ)
    nc = _prog_cache[RQ]

    res = bass_utils.run_bass_kernel_spmd(
        nc, in_maps, core_ids=list(range(N_CORES))
    )
    global _last_results
    _last_results = res
    outs = []
    for k in range(N_CORES):
        oT = res.results[k]["outT"]  # [32, NPAD]
        outs.append(np.ascontiguousarray(oT.T[:NPC]).astype(np.float32))
    return np.concatenate(outs, axis=0)


# revision 7
# speedup vs baseline: 1.0020x; 1.0020x over previous
"""GCNConv (DGL GraphConv norm='both') on 8 Trainium2 NeuronCores — v2.

out = D_dst^-1/2 * A * (D_src^-1/2 * X * W) + b
  X: [100000, 32] f32, edge_index: [2, 1600000] (src, dst), W: [32, 32], b: [32]

v2 design (vs v1 baseline):
  - NO collective: aggregation is linear, so aggregate RAW scaled features
    x_hat = x * outdeg^-1/2 (f16) and apply W AFTER aggregation. Every core
    computes the full x_hat table (x load is cheap) into its OWN dram — the
    283us AllGather is gone, as are phase-1 transposes/matmuls.
  - Compact f16 message table [100000, 32] (64B rows). dma_gather elements
    must be 256B, so gather fetches node QUADS (4 rows); edges are bucketed
    by (dst window, src%4) so each 128-edge block reads its message at a
    static 32-column slice of the quad.
  - One merged gather stream per span (not per group) -> fewer SWDGE calls.
  - One-hot build split DVE/Pool to balance engine load.
  - Transform-last per window: agg[128,32] -(x nd, ACT)-> f16 -(PE transpose)
    -> [32,128] -(x W, PE)-> psum, bias on DVE, batched DMA of 4 windows.
  - Output written transposed [32, 12544]; host untransposes (layout only).
"""

import os
import sys

import numpy as np

for _p in ("/opt/trn_rl_repo", "/root/.axon_site/_ro/trn_rl_repo"):
    if os.path.isdir(_p) and _p not in sys.path:
        sys.path.insert(0, _p)

N_NODES = 100000
N_CORES = 8
NPC = N_NODES // N_CORES  # 12500
DIN = 32
DOUT = 32
P = 128
NTILE = (NPC + P - 1) // P  # 98 dst windows/core
NPAD = NTILE * P            # 12544
NG = 4                      # src mod-4 groups
QN = N_NODES // NG          # 25000 quads

GMAIN = 781                 # nodes per partition in phase-1 main region
NMAIN = P * GMAIN           # 99968
NTAIL = N_NODES - NMAIN     # 32
GCH = 52                    # phase-1 chunk cols (11 chunks: 10*71+71=781)

SPAN_W = 8                  # windows per gather span


def _build_program(RQ):
    from concourse import bacc, bass, mybir, tile

    f32 = mybir.dt.float32
    f16 = mybir.dt.float16
    i16 = mybir.dt.int16
    i32 = mybir.dt.int32
    Alu = mybir.AluOpType
    Act = mybir.ActivationFunctionType

    WSL = NG * RQ               # slots (128-edge blocks) per window
    SLOTS = NTILE * WSL
    # SBUF-bounded gather span: 8 windows at RQ=5 (tuned); fewer if RQ grows
    span_w = max(2, (SPAN_W * 20) // WSL)
    nspan = (NTILE + span_w - 1) // span_w

    nc = bacc.Bacc(
        "TRN2",
        target_bir_lowering=False,
        debug=False,
        enable_asserts=False,
        num_devices=N_CORES,
    )

    # ---- I/O ----
    x_in = nc.dram_tensor("x_in", [N_NODES, DIN], f32, kind="ExternalInput")
    w_in = nc.dram_tensor("w_in", [DIN, DOUT], f32, kind="ExternalInput")
    b_in = nc.dram_tensor("b_in", [DOUT, 1], f32, kind="ExternalInput")
    qidx = nc.dram_tensor("qidx", [P, SLOTS * 8], i16, kind="ExternalInput")
    dstloc = nc.dram_tensor("dstloc", [P, SLOTS], f16, kind="ExternalInput")
    odeg_in = nc.dram_tensor("odeg_in", [P, GMAIN + 1], i32, kind="ExternalInput")
    ideg_in = nc.dram_tensor("ideg_in", [P, NTILE], i32, kind="ExternalInput")
    iota_in = nc.dram_tensor("iota_in", [P, P], f16, kind="ExternalInput")
    ident_in = nc.dram_tensor("ident_in", [P, P], f32, kind="ExternalInput")
    outT = nc.dram_tensor("outT", [DOUT, NPAD], f16, kind="ExternalOutput")

    # own-DRAM message table (f16, node-major rows of 32)
    m_dram = nc.dram_tensor("m_dram", [N_NODES * DIN], f16, kind="Internal")

    with tile.TileContext(nc) as tc:
        with (
            tc.tile_pool(name="const", bufs=1) as cpool,
            tc.tile_pool(name="p1", bufs=4) as p1pool,
            tc.tile_pool(name="work", bufs=3) as wpool,
            tc.tile_pool(name="nrm", bufs=1) as npool,
            tc.tile_pool(name="ohp", bufs=3) as ohpool,
            tc.tile_pool(name="gath", bufs=2) as gpool,
            tc.tile_pool(name="psA", bufs=4, space="PSUM") as ppa,
            tc.tile_pool(name="psB", bufs=2, space="PSUM") as ppb,
            tc.tile_pool(name="psC", bufs=2, space="PSUM") as ppc,
        ):
            # ---- degree tables first: norms ready before phase-1 ----
            od_t = cpool.tile([P, GMAIN + 1], i32)
            nc.sync.dma_start(out=od_t[:], in_=odeg_in[:])
            id_t = cpool.tile([P, NTILE], i32)
            nc.sync.dma_start(out=id_t[:], in_=ideg_in[:])

            ns_all = cpool.tile([P, GMAIN + 1], f32)  # outdeg^-1/2 (phase-1 layout)
            odf = npool.tile([P, GMAIN + 1], f32, tag="odf")
            nc.vector.tensor_copy(out=odf[:], in_=od_t[:])
            nc.vector.tensor_scalar_max(out=odf[:], in0=odf[:], scalar1=1.0)
            osq = npool.tile([P, GMAIN + 1], f32, tag="osq")
            nc.scalar.activation(out=osq[:], in_=odf[:], func=Act.Sqrt)
            nc.vector.reciprocal(out=ns_all[:], in_=osq[:])

            nd_all = cpool.tile([P, NTILE], f32)  # indeg^-1/2 (dst-lane layout)
            idf = npool.tile([P, NTILE], f32, tag="idf")
            nc.vector.tensor_copy(out=idf[:], in_=id_t[:])
            nc.vector.tensor_scalar_max(out=idf[:], in0=idf[:], scalar1=1.0)
            isq = npool.tile([P, NTILE], f32, tag="isq")
            nc.scalar.activation(out=isq[:], in_=idf[:], func=Act.Sqrt)
            nc.vector.reciprocal(out=nd_all[:], in_=isq[:])

            # ---- constants ----
            qidx_t = cpool.tile([P, SLOTS * 8], i16)
            dst_t = cpool.tile([P, SLOTS], f16)
            nc.sync.dma_start(out=dst_t[:], in_=dstloc[:])
            iota_t = cpool.tile([P, P], f16)
            nc.sync.dma_start(out=iota_t[:], in_=iota_in[:])
            ident_t = cpool.tile([P, P], f32)
            nc.sync.dma_start(out=ident_t[:], in_=ident_in[:])
            wf_t = cpool.tile([DIN, DOUT], f32)
            nc.sync.dma_start(out=wf_t[:], in_=w_in[:])
            w16 = cpool.tile([DIN, DOUT], f16)
            nc.scalar.activation(out=w16[:], in_=wf_t[:], func=Act.Copy)
            b_t = cpool.tile([DOUT, 1], f32)
            nc.sync.dma_start(out=b_t[:], in_=b_in[:])

            # ---- phase 1: x_hat = x * ns -> f16 table, replicated ----
            x_main = x_in[0:NMAIN, :].rearrange("(p g) c -> p g c", p=P)
            m_main = m_dram[0:NMAIN * DIN].rearrange(
                "(p g c) -> p g c", p=P, g=GMAIN
            )
            # tail 32 nodes first (keeps the final m write off the
            # phase-1 -> phase-2 critical path)
            xs2 = p1pool.tile([NTAIL, DIN], f32, tag="xs2")
            nc.sync.dma_start(out=xs2[:], in_=x_in[NMAIN:N_NODES, :])
            xh2 = p1pool.tile([NTAIL, DIN], f16, tag="xh2")
            nc.vector.tensor_tensor(
                out=xh2[:], in0=xs2[:],
                in1=ns_all[0:NTAIL, GMAIN:GMAIN + 1].to_broadcast([NTAIL, DIN]),
                op=Alu.mult,
            )
            nc.sync.dma_start(
                out=m_dram[NMAIN * DIN:].rearrange("(p c) -> p c", p=NTAIL),
                in_=xh2[:],
            )
            n_chunks = (GMAIN + GCH - 1) // GCH
            qsl = (SLOTS * 8 + n_chunks - 1) // n_chunks
            for ci, c0 in enumerate(range(0, GMAIN, GCH)):
                # slice of the big index table: fills DMA bubbles while the
                # phase-1 write waits on the DVE scale
                q0 = ci * qsl
                q1 = min(q0 + qsl, SLOTS * 8)
                if q0 < q1:
                    nc.sync.dma_start(out=qidx_t[:, q0:q1], in_=qidx[:, q0:q1])
                ncg = min(GCH, GMAIN - c0)
                xs = p1pool.tile([P, GCH, DIN], f32, tag="xs")
                nc.sync.dma_start(out=xs[:, :ncg, :], in_=x_main[:, c0:c0 + ncg, :])
                xh = p1pool.tile([P, GCH, DIN], f16, tag="xh")
                nc.vector.tensor_tensor(
                    out=xh[:, :ncg, :], in0=xs[:, :ncg, :],
                    in1=ns_all[:, c0:c0 + ncg].unsqueeze(2)
                    .to_broadcast([P, ncg, DIN]),
                    op=Alu.mult,
                )
                nc.sync.dma_start(out=m_main[:, c0:c0 + ncg, :], in_=xh[:, :ncg, :])

            # ---- phase 2 ----
            m_q = m_dram[:].rearrange("(q e) -> q e", e=P)  # [25000, 128] f16

            q_tiles = [None] * nspan

            def ensure_span(sp):
                if q_tiles[sp] is not None:
                    return
                w0 = sp * span_w
                nw = min(span_w, NTILE - w0)
                qt = gpool.tile([P, span_w * WSL, P], f16, tag="qt")
                # chunk gathers: keep num_idxs within the SWDGE desc ring;
                # last span goes window-by-window to shorten the tail drain
                step = 1 if sp in (0, nspan - 1) else 2
                for wc in range(0, nw, step):
                    nwc = min(step, nw - wc)
                    s0 = (w0 + wc) * WSL
                    nsl = nwc * WSL
                    n_idx = nsl * P
                    nc.gpsimd.dma_gather(
                        out_ap=qt[:, wc * WSL:wc * WSL + nsl, :],
                        in_ap=m_q[:],
                        idxs_ap=qidx_t[:, s0 * 8:(s0 + nsl) * 8],
                        num_idxs=n_idx,
                        num_idxs_reg=n_idx,
                        elem_size=P,
                        single_packet=False,
                    )
                q_tiles[sp] = qt

            LOOKAHEAD = 2
            oh_pending = {}

            def emit_onehot(wn):
                # one-hot [e-lane, slot, d]; sentinel dst=128 matches nothing
                oht = ohpool.tile([P, WSL, P], f16, tag="oh")
                nc.vector.tensor_tensor(
                    out=oht[:],
                    in0=iota_t[:].unsqueeze(1).to_broadcast([P, WSL, P]),
                    in1=dst_t[:, wn * WSL:(wn + 1) * WSL]
                    .unsqueeze(2).to_broadcast([P, WSL, P]),
                    op=Alu.is_equal,
                )
                oh_pending[wn] = oht

            for wn in range(min(LOOKAHEAD, NTILE)):
                emit_onehot(wn)

            for w in range(NTILE):
                sp, wo = divmod(w, span_w)
                ensure_span(sp)
                if sp + 1 < nspan and wo == 0:
                    ensure_span(sp + 1)
                if w + LOOKAHEAD < NTILE:
                    emit_onehot(w + LOOKAHEAD)
                oh = oh_pending.pop(w)
                ps = ppa.tile([P, DOUT], f32)
                qt = q_tiles[sp]
                for s in range(WSL):
                    g = s // RQ
                    nc.tensor.matmul(
                        out=ps[:],
                        lhsT=oh[:, s, :],
                        rhs=qt[:, wo * WSL + s, g * DIN:(g + 1) * DIN],
                        start=(s == 0),
                        stop=(s == WSL - 1),
                    )
                if wo == span_w - 1 or w == NTILE - 1:
                    q_tiles[sp] = None

                # scale by nd, cast f32 (ACT), transpose (PE), cast f16
                agg_sb = wpool.tile([P, DOUT], f32, tag="agg")
                nc.scalar.activation(
                    out=agg_sb[:], in_=ps[:], func=Act.Copy,
                    scale=nd_all[:, w:w + 1],
                )
                pst = ppb.tile([DOUT, P], f32)
                nc.tensor.transpose(out=pst[:], in_=agg_sb[:], identity=ident_t[:])
                aggT = wpool.tile([DOUT, P], f16, tag="aggT")
                nc.scalar.activation(out=aggT[:], in_=pst[:], func=Act.Copy)

                # resT[c',d] = sum_c W[c,c'] aggT[c,d]
                r4 = w % 4
                if r4 == 0:
                    res4 = ppc.tile([DOUT, 4, P], f32)
                nc.tensor.matmul(
                    out=res4[:, r4, :], lhsT=w16[:], rhs=aggT[:],
                    start=True, stop=True,
                )
                if r4 == 3 or w == NTILE - 1:
                    nb = r4 + 1
                    res_sb = wpool.tile([DOUT, 4, P], f16, tag="res")
                    nc.vector.tensor_tensor(
                        out=res_sb[:, :nb, :],
                        in0=res4[:, :nb, :],
                        in1=b_t[:].unsqueeze(2).to_broadcast([DOUT, nb, P]),
                        op=Alu.add,
                    )
                    w0 = w - nb + 1
                    nc.sync.dma_start(
                        out=outT[:, w0 * P:(w + 1) * P].rearrange(
                            "c (n p) -> c n p", p=P
                        ),
                        in_=res_sb[:, :nb, :],
                    )

    nc.compile()
    return nc


def _preprocess(x, edge_index, W, b):
    """Host-side: integer bucketing/sorting only (+ layout copies)."""
    src = np.asarray(edge_index[0], dtype=np.int64)
    dst = np.asarray(edge_index[1], dtype=np.int64)
    x = np.asarray(x, dtype=np.float32)
    W = np.asarray(W, dtype=np.float32)
    b = np.asarray(b, dtype=np.float32)

    core_of = dst // NPC
    per_core = []
    rq_needed = 1
    for k in range(N_CORES):
        sel = core_of == k
        s_k = src[sel]
        d_k = dst[sel] - k * NPC
        win = d_k >> 7
        grp = s_k & 3
        order = np.lexsort((s_k, grp, win))
        s_k = s_k[order]
        d_k = d_k[order]
        wg = win[order] * NG + grp[order]
        wg_counts = np.bincount(wg, minlength=NTILE * NG)
        rq_needed = max(rq_needed, int(np.ceil(wg_counts.max() / P)))
        ideg = np.bincount(d_k, minlength=NPC)
        per_core.append((s_k, d_k, wg_counts, ideg))

    RQ = int(rq_needed)
    WSL = NG * RQ
    SLOTS = NTILE * WSL

    odeg_full = np.bincount(src, minlength=N_NODES).astype(np.int64)
    odeg_arr = np.zeros((P, GMAIN + 1), dtype=np.int32)
    odeg_arr[:, :GMAIN] = odeg_full[:NMAIN].reshape(P, GMAIN)
    odeg_arr[:NTAIL, GMAIN] = odeg_full[NMAIN:]

    iota_rep = np.broadcast_to(
        np.arange(P, dtype=np.float16)[None, :], (P, P)
    ).copy()
    ident = np.eye(P, dtype=np.float32)

    in_maps = []
    for k in range(N_CORES):
        s_k, d_k, wg_counts, ideg = per_core[k]
        wg_starts = np.concatenate([[0], np.cumsum(wg_counts)])[:-1]
        n_e = len(s_k)
        pos = np.arange(n_e) - np.repeat(wg_starts, wg_counts)
        wv = np.repeat(np.arange(NTILE * NG) // NG, wg_counts)
        gv = np.repeat(np.arange(NTILE * NG) % NG, wg_counts)
        js = (wv * WSL + gv * RQ) * P + pos  # stream position, lane-fastest

        qflat = np.zeros(SLOTS * P, dtype=np.int16)
        qflat[js] = (s_k >> 2).astype(np.int16)
        dflat = np.full(SLOTS * P, P, dtype=np.float16)
        dflat[js] = (d_k & 127).astype(np.float16)

        qi = qflat.reshape(SLOTS * P // 16, 16).T  # [16, SLOTS*8]
        qidx_arr = np.tile(qi, (8, 1))
        dst_arr = dflat.reshape(SLOTS, P).T.copy()

        ideg_pad = np.zeros(NPAD, dtype=np.int32)
        ideg_pad[:NPC] = ideg
        ideg_arr = ideg_pad.reshape(NTILE, P).T.copy()

        in_maps.append({
            "x_in": x, "w_in": W, "b_in": b[:, None].copy(),
            "qidx": qidx_arr, "dstloc": dst_arr,
            "odeg_in": odeg_arr, "ideg_in": ideg_arr,
            "iota_in": iota_rep, "ident_in": ident,
        })

    return in_maps, RQ


_prog_cache = {}
_last_results = None


def kernel(x, edge_index, W, b):
    from concourse import bass_utils

    in_maps, RQ = _preprocess(x, edge_index, W, b)
    if RQ not in _prog_cache:
        _prog_cache[RQ] = _build_program(RQ)
    nc = _prog_cache[RQ]

    res = bass_utils.run_bass_kernel_spmd(
        nc, in_maps, core_ids=list(range(N_CORES))
    )
    global _last_results
    _last_results = res
    outs = []
    for k in range(N_CORES):
        oT = res.results[k]["outT"]  # [32, NPAD]
        outs.append(np.ascontiguousarray(oT.T[:NPC]).astype(np.float32))
    return np.concatenate(outs, axis=0)
